# revision 1
# baseline (speedup 1.0000x reference)
"""Paged attention with RoPE (decode, B=16, L=1) on 8 trn2 NeuronCores.

Sharding: tensor-parallel over heads. 32 heads / 8 cores = 4 heads per core.
Per core: QKV projection for its heads, RoPE, paged attention over its head
shard of the kv cache, partial output projection; host sums the 8 partials.

Program order is engineered around the per-engine FIFO streams so the DMA
engines (the roofline resource; ~70 MB/core) stay saturated:
  q projection -> paged K gathers + scores (overlap the k/v weight stream)
  -> k projection -> new-token score patches -> v projection
  -> softmax + V phase in two waves of 8 sequences -> output projection.
The reference's reshape_and_cache scatter is never materialized: positions
whose cache slot the new tokens overwrite get their K score patched from a
small q.k_new matrix (16-byte DMAs into the score tile), and their V rows
are zeroed in the gathered tile with the new-token contribution added as a
rank-1 correction matmul after softmax.
"""

import numpy as np

B = 16
HID = 4096
NH = 32
HD = 128
BS = 16
MAXB = 64
NB = 1024
S = MAXB * BS          # 1024 max context
NSLOT = NB * BS        # 16384
N_CORES = 8
HPC = NH // N_CORES    # 4 heads per core
HDPC = HPC * HD        # 512 elements of head-dim per core
KTILES = HID // 128    # 32
NOUT = HID // 512      # 8 output-projection chunks
SCALE = 1.0 / float(np.sqrt(HD))
NEG = -1.0e30


def _plan(position_ids, block_tables, slots, context_lengths, cos_sin_cache):
    """Host-side planning shared by all cores: gather indices, fixups, masks,
    rope tables, per-sequence tile counts."""
    pos = np.clip(position_ids.reshape(B).astype(np.int64), 0,
                  cos_sin_cache.shape[0] - 1)
    bt = block_tables.astype(np.int64)          # [B, MAXB]
    sl = slots.astype(np.int64)                 # [B]
    ctx = context_lengths.astype(np.int64)      # [B]

    # padded gather length: multiple of 128; ctx==0 -> full S, uniform probs
    ctx_eff = np.maximum(ctx, 1)
    pad_len = ((ctx_eff + 127) // 128 * 128).astype(np.int64)
    pad_len = np.where(ctx == 0, S, pad_len)
    nj = (pad_len // 128).astype(np.int64)
    max_nj = int(nj.max())

    p_all = np.arange(S, dtype=np.int64)
    slot_all = bt[:, p_all // BS] * BS + (p_all % BS)      # [B, S]

    # new-token overwrite map: slot -> writing sequence (last writer wins)
    fix = np.full((NSLOT,), -1, dtype=np.int64)
    for bp in range(B):
        fix[sl[bp]] = bp

    fixups = []   # per b: list of (gathered position p, source sequence b')
    for b in range(B):
        n = int(pad_len[b])
        lim = int(min(ctx[b], n)) if ctx[b] > 0 else S
        fb = [(int(p), int(fix[slot_all[b, p]]))
              for p in range(lim) if fix[slot_all[b, p]] >= 0]
        fixups.append(fb)

    # int16 gather index tiles, wrapped mod 16 partitions, replicated x8
    idx_tiles = np.zeros((B, 128, S // 16), dtype=np.int16)
    for b in range(B):
        n = int(pad_len[b])
        idx = slot_all[b, :n].copy()
        lim = int(min(ctx[b], n)) if ctx[b] > 0 else S
        idx[lim:] = idx[0]                     # pad with a valid row
        wrapped = idx.reshape(n // 16, 16).T   # [16, n/16]
        idx_tiles[b, :, : n // 16] = np.tile(wrapped, (8, 1))

    # additive mask [64 rows = b*HPC+h, max context]
    mask = np.zeros((64, S), dtype=np.float32)
    for b in range(B):
        if ctx[b] > 0:
            mask[b * HPC:(b + 1) * HPC, int(ctx[b]):] = NEG
        else:
            mask[b * HPC:(b + 1) * HPC, :] = NEG   # uniform over full S
    mask = np.ascontiguousarray(mask[:, :max_nj * 128])

    # rope tables, per-head replicated, sin sign baked ( [-sin | +sin] )
    cs = cos_sin_cache[pos]                     # [B, 128]
    cos_h, sin_h = cs[:, :64], cs[:, 64:]
    cos_full = np.concatenate([cos_h, cos_h], axis=1)
    sin_sign = np.concatenate([-sin_h, sin_h], axis=1)
    cos_rep = np.ascontiguousarray(np.tile(cos_full, (1, HPC)), dtype=np.float32)
    sin_rep = np.ascontiguousarray(np.tile(sin_sign, (1, HPC)), dtype=np.float32)

    return {
        'nj': [int(x) for x in nj], 'max_nj': max_nj, 'fixups': fixups,
        'idx_tiles': idx_tiles, 'mask': mask,
        'cos_rep': cos_rep, 'sin_rep': sin_rep,
    }


def _build_bass(plan):
    """Build the per-core bass program (identical program for every core;
    only the input data differs)."""
    import concourse.tile as tile
    from concourse import bacc, mybir
    from concourse.masks import make_identity
    from contextlib import ExitStack

    fp32 = mybir.dt.float32
    f32r = mybir.dt.float32r
    i16 = mybir.dt.int16
    AX = mybir.AxisListType
    ALU = mybir.AluOpType
    ACTF = mybir.ActivationFunctionType

    nj = plan['nj']
    max_nj = plan['max_nj']
    fixups = plan['fixups']
    SW = max_nj * 128          # score width
    n_fix = sum(len(fb) for fb in fixups)

    nc = bacc.Bacc("TRN2", target_bir_lowering=False, debug=False,
                   num_devices=N_CORES)

    hiddenT = nc.dram_tensor("hiddenT", [HID, B], fp32, kind="ExternalInput")
    wqkvT = nc.dram_tensor("wqkvT", [HID, 3 * HDPC], fp32, kind="ExternalInput")
    woT = nc.dram_tensor("woT", [HDPC, HID], fp32, kind="ExternalInput")
    ksrc = nc.dram_tensor("ksrc", [NSLOT, HDPC], fp32, kind="ExternalInput")
    vsrc = nc.dram_tensor("vsrc", [NSLOT, HDPC], fp32, kind="ExternalInput")
    idxs = nc.dram_tensor("idxs", [B, 128, S // 16], i16, kind="ExternalInput")
    maskd = nc.dram_tensor("maskd", [64, SW], fp32, kind="ExternalInput")
    cosd = nc.dram_tensor("cosd", [B, HDPC], fp32, kind="ExternalInput")
    sind = nc.dram_tensor("sind", [B, HDPC], fp32, kind="ExternalInput")
    y = nc.dram_tensor("y", [B, HID], fp32, kind="ExternalOutput")

    with tile.TileContext(nc) as tc, ExitStack() as ctx:
        const_p = ctx.enter_context(tc.tile_pool(name="const", bufs=1))
        w_p = ctx.enter_context(tc.tile_pool(name="w", bufs=6))
        kv_p = ctx.enter_context(tc.tile_pool(name="kv", bufs=3))
        vg_p = ctx.enter_context(tc.tile_pool(name="vgp", bufs=3))
        sb_p = ctx.enter_context(tc.tile_pool(name="sb", bufs=1))
        tmp_p = ctx.enter_context(tc.tile_pool(name="tmp", bufs=2))
        psacc = ctx.enter_context(tc.tile_pool(name="psacc", bufs=2, space="PSUM"))
        pssm = ctx.enter_context(tc.tile_pool(name="pssm", bufs=1, space="PSUM"))
        psat = ctx.enter_context(tc.tile_pool(name="psat", bufs=1, space="PSUM"))
        psyt = ctx.enter_context(tc.tile_pool(name="psyt", bufs=1, space="PSUM"))
        psqr = ctx.enter_context(tc.tile_pool(name="psqr", bufs=1, space="PSUM"))

        # ---------------- constants ----------------
        ident = const_p.tile([128, 128], fp32)
        make_identity(nc, ident[:])
        zrow = const_p.tile([1, HDPC], fp32)
        nc.vector.memset(zrow[:], 0)
        ht_sb = const_p.tile([128, KTILES * B], fp32)
        nc.sync.dma_start(ht_sb[:].rearrange("p (t b) -> p t b", b=B),
                            hiddenT.ap().rearrange("(t p) b -> p t b", p=128))
        cos_sb = const_p.tile([B, HDPC], fp32)
        nc.sync.dma_start(cos_sb[:], cosd.ap())
        sin_sb = const_p.tile([B, HDPC], fp32)
        nc.sync.dma_start(sin_sb[:], sind.ap())
        mask_sb = const_p.tile([32, 2 * SW], fp32)
        nc.sync.dma_start(mask_sb[:].rearrange("p (g w) -> p g w", g=2),
                          maskd.ap().rearrange("(g p) w -> p g w", g=2))
        idx_sb = const_p.tile([128, B * (S // 16)], i16)
        nc.sync.dma_start(idx_sb[:].rearrange("p (b c) -> p b c", b=B),
                          idxs.ap().rearrange("b p c -> p b c"))

        def rope(dst, src):
            src3 = src.rearrange("b (h two d) -> b h two d", two=2, d=64)
            rot = tmp_p.tile([B, HDPC], fp32, name="rot", tag="rot", bufs=1)
            rot3 = rot[:].rearrange("b (h two d) -> b h two d", two=2, d=64)
            nc.vector.tensor_copy(rot3[:, :, 0, :], src3[:, :, 1, :])
            nc.vector.tensor_copy(rot3[:, :, 1, :], src3[:, :, 0, :])
            nc.vector.tensor_mul(rot[:], rot[:], sin_sb[:])
            cp = tmp_p.tile([B, HDPC], fp32, name="cp", tag="cp", bufs=1)
            nc.vector.tensor_mul(cp[:], src, cos_sb[:])
            nc.vector.tensor_add(dst[:], cp[:], rot[:])

        q_sb = sb_p.tile([B, HDPC], fp32)
        k_sb = sb_p.tile([B, HDPC], fp32)
        v_sb = sb_p.tile([B, HDPC], fp32)

        def wpass(col0, out_ps):
            for kt in range(KTILES):
                wt = w_p.tile([128, HPC * 512], fp32, name="wt", tag="w",
                              padded_shape=[128, HPC * 512])
                nc.sync.dma_start(wt[:, :HDPC],
                                    wqkvT.ap()[kt * 128:(kt + 1) * 128,
                                               col0:col0 + HDPC])
                nc.tensor.matmul(out_ps[:],
                                 ht_sb[:, kt * B:(kt + 1) * B],
                                 wt[:, :HDPC],
                                 start=(kt == 0), stop=(kt == KTILES - 1))

        # ---------------- q projection (first, to unblock scores) ----------
        q_ps = pssm.tile([B, HDPC], fp32, name="q_ps", tag="sm")
        wpass(0, q_ps)
        rope(q_sb, q_ps[:])

        # ---------------- K gather + scores ----------------
        swide = sb_p.tile([128, max_nj * 64], fp32)
        nc.gpsimd.memset(swide[:], 0)

        def qrep_bcast(b):
            # qrep = row b of q_sb on all partitions: eye-column broadcast
            # matmul (lhsT[k, m] = ident[k, b] for every m).
            qr_ps = psqr.tile([128, HDPC], fp32, name="qr_ps", tag="qr")
            nc.tensor.matmul(qr_ps[:],
                             ident[:B, b:b + 1].to_broadcast([B, 128])
                             ,
                             q_sb[:], start=True, stop=True)
            qrep = tmp_p.tile([128, HDPC], fp32, name="qrep", tag="qrep",
                              bufs=4)
            nc.scalar.copy(qrep[:], qr_ps[:])
            return qrep

        def score_tile(b, j, src, qrep):
            prod = tmp_p.tile([128, HDPC], fp32, name="prod", tag="prod")
            nc.vector.tensor_mul(prod[:], src, qrep[:])
            nc.vector.tensor_reduce(
                out=swide[:, j * 64 + b * HPC: j * 64 + (b + 1) * HPC],
                in_=prod[:].rearrange("p (h d) -> p h d", d=HD),
                axis=AX.X, op=ALU.add)

        for b in range(B):
            n = nj[b] * 128
            kg = kv_p.tile([128, max_nj * HDPC], fp32, name="kg", tag="kvg")
            nc.gpsimd.dma_gather(
                out_ap=kg[:].rearrange("p (j e) -> p j e", e=HDPC)[:, :nj[b], :],
                in_ap=ksrc.ap(),
                idxs_ap=idx_sb[:, b * (S // 16): b * (S // 16) + n // 16],
                num_idxs=n, num_idxs_reg=n, elem_size=HDPC)
            qrep = qrep_bcast(b)
            for j in range(nj[b]):
                score_tile(b, j, kg[:, j * HDPC:(j + 1) * HDPC], qrep)

        # ---------------- k projection + new-token score patches -----------
        k_ps = pssm.tile([B, HDPC], fp32, name="k_ps", tag="sm")
        wpass(HDPC, k_ps)
        rope(k_sb, k_ps[:])

        # F[b', 4b+h] = q_{b,h} . k_new_{b',h} via four [16,16] matmuls on
        # transposed q/k slices; patch affected swide cells with 16B DMAs.
        if n_fix:
            qkT = sb_p.tile([128, 2 * HPC * B], fp32)   # [d, (qk, h, b)]
            for s_i, src in ((0, q_sb), (1, k_sb)):
                for h in range(HPC):
                    tp = pssm.tile([128, B], fp32, name="tp", tag="sm")
                    nc.tensor.transpose(tp[:], src[:, h * HD:(h + 1) * HD],
                                        ident[:B, :B])
                    nc.scalar.copy(
                        qkT[:, (s_i * HPC + h) * B:(s_i * HPC + h + 1) * B],
                        tp[:])
            F_ps = pssm.tile([B, HPC * B], fp32, name="F_ps", tag="sm")
            for h in range(HPC):
                nc.tensor.matmul(F_ps[:, h:HPC * B:HPC],
                                 qkT[:, (HPC + h) * B:(HPC + h + 1) * B],
                                 qkT[:, h * B:(h + 1) * B],
                                 start=True, stop=True)
            F_sb = sb_p.tile([B, HPC * B], fp32)
            nc.scalar.copy(F_sb[:], F_ps[:])
            for b in range(B):
                for (p, bp) in fixups[b]:
                    nc.sync.dma_start(
                        swide[p % 128:p % 128 + 1,
                              (p // 128) * 64 + b * HPC:
                              (p // 128) * 64 + (b + 1) * HPC],
                        F_sb[bp:bp + 1, b * HPC:(b + 1) * HPC])

        # ---------------- v projection ----------------
        v_ps = pssm.tile([B, HDPC], fp32, name="v_ps", tag="sm")
        wpass(2 * HDPC, v_ps)
        nc.vector.tensor_copy(v_sb[:], v_ps[:])

        # wo prefetch (slot rotation lets these stream in during the V phase)
        wo_tiles = []
        for i in range(NOUT):
            if i < NOUT - 2:
                wo = w_p.tile([128, HPC * 512], fp32, name="wo", tag="w")
            else:
                wo = kv_p.tile([128, HPC * 512], fp32, name="wo", tag="kvg")
            nc.sync.dma_start(
                wo[:].rearrange("p (t c) -> p t c", t=HPC),
                woT.ap()[:, i * 512:(i + 1) * 512]
                .rearrange("(t p) c -> p t c", p=128))
            wo_tiles.append(wo)

        # -------- softmax + V phase, two waves of 8 sequences each ---------
        pT = sb_p.tile([128, max_nj * 64], fp32)
        p_waves = []
        atT_ps = psat.tile([128, 64], fp32)      # cols h*16+b
        # p-scalars for the rank-1 new-token corrections, cell [b', b*4+h];
        # multiple fixups landing in the same column sum in the matmul, but a
        # duplicated (b, b') pair needs an overflow column of its own.
        flat_fixups = [(b, p, bp) for b in range(B) for (p, bp) in fixups[b]]
        seen, base_fix, extra_fix = set(), [], []
        for (b, p, bp) in flat_fixups:
            if (b, bp) in seen:
                extra_fix.append((b, p, bp))
            else:
                seen.add((b, bp))
                base_fix.append((b, p, bp))
        n_extra = len(extra_fix)
        psc = sb_p.tile([B, 4 * B + 4 * max(1, n_extra)], fp32)
        nc.vector.memset(psc[:], 0)

        for g in range(2):
            r0, r1 = 32 * g, 32 * (g + 1)        # bh rows of this wave
            sc_ps = psacc.tile([32, SW], fp32, name="sc_ps", tag="acc")
            for j in range(max_nj):
                nc.tensor.transpose(
                    sc_ps[:, j * 128:(j + 1) * 128],
                    swide[:, j * 64 + r0: j * 64 + r1], ident[:])
            sc = sb_p.tile([32, SW], fp32, name="sc", tag="sc", bufs=1)
            nc.vector.scalar_tensor_tensor(
                out=sc[:], in0=sc_ps[:], scalar=SCALE,
                in1=mask_sb[:, g * SW:(g + 1) * SW], op0=ALU.mult, op1=ALU.add)
            negmax = sb_p.tile([32, 1], fp32, name="negmax", tag="nm", bufs=2)
            nc.vector.tensor_reduce(out=negmax[:], in_=sc[:],
                                    axis=AX.X, op=ALU.max, negate=True)
            p_sb = sb_p.tile([32, SW], fp32, name="p_sb", tag=f"pw{g}")
            sums = sb_p.tile([32, 1], fp32, name="sums", tag="sums", bufs=2)
            nc.scalar.activation(out=p_sb[:], in_=sc[:],
                                 func=ACTF.Exp, bias=negmax[:],
                                 scale=1.0, accum_out=sums[:])
            rsum = sb_p.tile([32, 1], fp32, name="rsum", tag="rs", bufs=2)
            nc.vector.reciprocal(rsum[:], sums[:])
            nc.vector.tensor_scalar_mul(p_sb[:], p_sb[:], rsum[:])
            p_waves.append(p_sb)
            for j in range(max_nj):
                pt_ps = pssm.tile([128, 32], fp32, name="pt_ps", tag="sm")
                nc.tensor.transpose(pt_ps[:], p_sb[:, j * 128:(j + 1) * 128],
                                    ident[:32, :32])
                nc.scalar.copy(pT[:, j * 64 + r0: j * 64 + r1], pt_ps[:])

            # p-scalars for this wave's rank-1 corrections
            for (b, p, bp) in base_fix:
                if 8 * g <= b < 8 * (g + 1):
                    nc.sync.dma_start(
                        psc[bp:bp + 1, 4 * b:4 * b + 4],
                        p_sb[(b - 8 * g) * HPC:(b - 8 * g + 1) * HPC, p:p + 1])
            for f, (b, p, bp) in enumerate(extra_fix):
                if 8 * g <= b < 8 * (g + 1):
                    nc.sync.dma_start(
                        psc[bp:bp + 1, 4 * B + 4 * f:4 * B + 4 * f + 4],
                        p_sb[(b - 8 * g) * HPC:(b - 8 * g + 1) * HPC, p:p + 1])

            for b in range(8 * g, 8 * g + 8):
                vg = vg_p.tile([128, max_nj * HDPC], fp32, name="vg", tag="vgt")
                n = nj[b] * 128
                nc.gpsimd.dma_gather(
                    out_ap=vg[:].rearrange("p (j e) -> p j e",
                                           e=HDPC)[:, :nj[b], :],
                    in_ap=vsrc.ap(),
                    idxs_ap=idx_sb[:, b * (S // 16): b * (S // 16) + n // 16],
                    num_idxs=n, num_idxs_reg=n, elem_size=HDPC)
                # zero the overwritten rows: their stale contribution must
                # vanish; the new-token term is added as a rank-1 correction.
                for (p, bp) in fixups[b]:
                    nc.sync.dma_start(
                        vg[p % 128:p % 128 + 1,
                           (p // 128) * HDPC:(p // 128 + 1) * HDPC], zrow[:])
                for h in range(HPC):
                    col = h * B + b
                    for j in range(nj[b]):
                        nc.tensor.matmul(
                            atT_ps[:, col:col + 1],
                            vg[:, j * HDPC + h * HD: j * HDPC + (h + 1) * HD]
                            ,
                            pT[:, j * 64 + b * HPC + h:
                               j * 64 + b * HPC + h + 1],
                            start=(j == 0), stop=(j == nj[b] - 1))

        # rank-1 corrections: attnT[:, h*16+b] += p[p*] * v_new[b'], computed
        # in a separate PSUM tile and folded in with the PSUM->SBUF move.
        attnT = sb_p.tile([128, 64], fp32)
        if flat_fixups:
            C_ps = pssm.tile([128, 64], fp32, name="C_ps", tag="sm")
            for h in range(HPC):
                nc.tensor.matmul(C_ps[:, h * B:(h + 1) * B],
                                 v_sb[:, h * HD:(h + 1) * HD],
                                 psc[:, h:4 * B:HPC],
                                 start=True, stop=True)
                for f, (b, p, bp) in enumerate(extra_fix):
                    nc.tensor.matmul(
                        C_ps[:, h * B + b: h * B + b + 1],
                        v_sb[:, h * HD:(h + 1) * HD],
                        psc[:, 4 * B + 4 * f + h: 4 * B + 4 * f + h + 1],
                        start=False, stop=True, skip_group_check=True)
            C_sb = sb_p.tile([128, 64], fp32)
            nc.scalar.copy(C_sb[:], C_ps[:])
            nc.vector.tensor_add(attnT[:], atT_ps[:], C_sb[:])
        else:
            nc.scalar.copy(attnT[:], atT_ps[:])

        # ------------- output projection (4 chunks packed per PE pass) ------
        for r in range(NOUT // 4):
            yt = psyt.tile([128, 512], fp32, name="yt", tag="yt")
            for c in range(4):
                i = r * 4 + c
                for h in range(HPC):
                    nc.tensor.matmul(yt[32 * c:32 * c + B, :],
                                     attnT[:, h * B:(h + 1) * B],
                                     wo_tiles[i][:, h * 512:(h + 1) * 512]
                                     ,
                                     start=(h == 0), stop=(h == HPC - 1),
                                     tile_position=(0, 32 * c))
            yst = tmp_p.tile([128, 512], fp32, name="yst", tag="yst")
            for c in range(4):
                nc.scalar.copy(yst[32 * c:32 * c + B, :], yt[32 * c:32 * c + B, :])
            for c in range(4):
                i = r * 4 + c
                nc.sync.dma_start(y.ap()[:, i * 512:(i + 1) * 512],
                                  yst[32 * c:32 * c + B, :])

    nc.compile()
    return nc


def _make_in_maps(hidden_states, qkv_w, out_w, key_cache, value_cache, plan):
    hid = hidden_states.reshape(B, HID).astype(np.float32)
    hiddenT = np.ascontiguousarray(hid.T)

    wq, wk, wv = qkv_w[:HID], qkv_w[HID:2 * HID], qkv_w[2 * HID:]
    kc = key_cache.reshape(NSLOT, NH, HD)
    vc = value_cache.reshape(NSLOT, NH, HD)

    in_maps = []
    for c in range(N_CORES):
        h0 = c * HPC
        r0, r1 = h0 * HD, (h0 + HPC) * HD
        wqkvT = np.ascontiguousarray(
            np.concatenate([wq[r0:r1], wk[r0:r1], wv[r0:r1]], axis=0).T)
        woT = np.ascontiguousarray(out_w[:, r0:r1].T)
        ks = np.ascontiguousarray(kc[:, h0:h0 + HPC, :].reshape(NSLOT, HDPC))
        vs = np.ascontiguousarray(vc[:, h0:h0 + HPC, :].reshape(NSLOT, HDPC))
        in_maps.append({
            "hiddenT": hiddenT, "wqkvT": wqkvT, "woT": woT,
            "ksrc": ks, "vsrc": vs,
            "idxs": plan['idx_tiles'], "maskd": plan['mask'],
            "cosd": plan['cos_rep'], "sind": plan['sin_rep'],
        })
    return in_maps


def kernel(hidden_states, qkv_w, out_w, cos_sin_cache, key_cache, value_cache,
           position_ids, block_tables, slots, context_lengths):
    from concourse.bass_utils import run_bass_kernel_spmd

    hidden_states = np.asarray(hidden_states, dtype=np.float32)
    qkv_w = np.asarray(qkv_w, dtype=np.float32)
    out_w = np.asarray(out_w, dtype=np.float32)
    cos_sin_cache = np.asarray(cos_sin_cache, dtype=np.float32)
    key_cache = np.asarray(key_cache, dtype=np.float32)
    value_cache = np.asarray(value_cache, dtype=np.float32)

    plan = _plan(np.asarray(position_ids), np.asarray(block_tables),
                 np.asarray(slots), np.asarray(context_lengths), cos_sin_cache)
    nc = _build_bass(plan)
    in_maps = _make_in_maps(hidden_states, qkv_w, out_w, key_cache,
                            value_cache, plan)

    res = run_bass_kernel_spmd(nc, in_maps, core_ids=list(range(N_CORES)))
    out = np.zeros((B, HID), dtype=np.float32)
    for c in range(N_CORES):
        out += res.results[c]["y"]
    return out.reshape(B, 1, HID)

